# revision 63
# baseline (speedup 1.0000x reference)
"""Trainium2 Bass kernel for nn_Distance (radius-graph + Distance, K=32 NN).

Strategy (data-parallel over molecules, 8 NeuronCores, single SPMD NEFF):
  - batch ids are sorted -> molecules are contiguous node ranges. The 128
    molecules are sorted by size and dealt round-robin: slot i of core c
    holds the (8*i + c)-th largest molecule, so per-slot padded widths
    (multiples of 32, shared by all cores) are tight and the NEFF is
    identical across cores; only the data differs.
  - Two <=64-node slots share one 128-partition tile (rows 0-63 / 64-127);
    larger slots get their own tile.
  - kmax = -d2 via one PE matmul per sub-slot (homogeneous trick:
    lhsT = [x; y; z; -1; sq], rhs = [2x; 2y; 2z; sq; -1], fp32).
  - Ordered top-32 per row on DVE: max8 / max_index / match_replace,
    4 rounds of 8; self edges killed by adding a -1e30 diagonal constant.
  - Neighbor coordinates recovered without per-edge DMA gathers: gpsimd
    local_scatter inverts the per-row index map, then scatters fp16 hi+lo
    coordinate plane pairs (PE ones-matmul broadcasts) into the 32 output
    slots; hi + lo reconstructs the fp32 coordinate exactly.
  - Weights / masks / global ids / edge vectors via consolidated
    [128, T*32] DVE/ACT ops; host only unpads + concatenates.
"""

import os

import numpy as np

N = 8192
NMOL = 128
K = 32
NCORES = 8
SLOTS = NMOL // NCORES  # 16 size-rank bands, one slot each
CUT_U2 = 25.0  # CUTOFF_UPPER ** 2
CUT_L = 0.5  # CUTOFF_LOWER
PADPOS = 1.0e4
NEGBIG = -1.0e30
SANITIZE_THRESH = -1.0e8  # kmax below this -> padded/diag slot

_BUILD_CACHE: dict = {}
VEC_EXACT = True  # hi+lo fp16 scatter pair -> f32-exact edge_vec


def _structure(slot_max_sizes):
    """Slot widths + tile packing from the per-slot max molecule size.

    Returns (SW, tiles): SW[i] = stored width of slot i (multiple of 32);
    tiles = list of ((slot, row_offset), ...) with 1 or 2 subs per tile.
    """
    SW = []
    for sm in slot_max_sizes:
        w = max(32, -(-int(sm) // 32) * 32)
        assert w <= 128, f"molecule with {sm} nodes does not fit one tile"
        SW.append(w)
    big = [i for i, w in enumerate(SW) if w > 64]
    small = [i for i, w in enumerate(SW) if w <= 64]
    for i in small:
        SW[i] = 64  # pairable slots stored 64 wide: two stack into 128 rows
    singles = [((i, 0),) for i in big]
    pairs = []
    for j in range(0, len(small), 2):
        pair = small[j : j + 2]
        pairs.append(tuple((s, 64 * p) for p, s in enumerate(pair)))
    # interleave pairs among singles for smoother engine rotation
    tiles = []
    si, pi = 0, 0
    while si < len(singles) or pi < len(pairs):
        for _ in range(2):
            if si < len(singles):
                tiles.append(singles[si]); si += 1
        if pi < len(pairs):
            tiles.append(pairs[pi]); pi += 1
    return tuple(SW), tuple(tiles)


def _build(struct_key):
    """Emit the SPMD Bass/Tile kernel for the given structure. Returns nc."""
    import contextlib

    import concourse.bacc as bacc
    import concourse.mybir as mybir
    import concourse.tile as tile

    SW, tiles = struct_key
    T = len(tiles)
    off_slot = np.concatenate([[0], np.cumsum(SW)]).astype(int)
    MW = int(off_slot[-1])
    TK = T * K

    nc = bacc.Bacc(
        "TRN2",
        target_bir_lowering=False,
        debug=False,
        num_devices=NCORES,
    )
    f32 = mybir.dt.float32
    f16 = mybir.dt.float16
    i32 = mybir.dt.int32
    i16 = mybir.dt.int16
    u16 = mybir.dt.uint16
    u8 = mybir.dt.uint8
    Alu = mybir.AluOpType

    d_posT = nc.dram_tensor("posT", [3, MW], f32, kind="ExternalInput")
    d_posTf = nc.dram_tensor("posTf", [1, 3 * MW], f16, kind="ExternalInput")
    d_posTflo = nc.dram_tensor("posTflo", [1, 3 * MW], f16, kind="ExternalInput")
    d_metaf = nc.dram_tensor("metaf", [128, 3 * T], f32, kind="ExternalInput")
    d_metai = nc.dram_tensor("metai", [128, 2 * T], i32, kind="ExternalInput")
    d_diagall = nc.dram_tensor("diagall", [128, 192], f32, kind="ExternalInput")

    d_oids = nc.dram_tensor("o_ids", [128, T, K, 2], i32, kind="ExternalOutput")
    d_owvec = nc.dram_tensor("o_wvec", [128, T, K, 4], f32, kind="ExternalOutput")
    d_omask = nc.dram_tensor("o_mask", [128, T, K], u8, kind="ExternalOutput")

    with tile.TileContext(nc) as tc:
        with (
            tc.tile_pool(name="persist", bufs=1) as pp,
            tc.tile_pool(name="work", bufs=3) as wp,
        ):
            # ---- load + prep shared planes ----
            posT = pp.tile([3, MW], f32)
            nc.sync.dma_start(posT, d_posT.ap())
            posTf = pp.tile([1, 3 * MW], f16)
            nc.sync.dma_start(posTf, d_posTf.ap())
            posTflo = pp.tile([1, 3 * MW], f16)
            if VEC_EXACT:
                nc.sync.dma_start(posTflo, d_posTflo.ap())
            diagall = pp.tile([128, 192], f32)
            nc.sync.dma_start(diagall, d_diagall.ap())
            diag = diagall[:, 0:128]
            diagp = diagall[:, 128:192]
            metaf = pp.tile([128, 3 * T], f32)
            nc.sync.dma_start(metaf, d_metaf.ap())
            cxyz = [metaf[:, c * T : (c + 1) * T] for c in range(3)]
            metai = pp.tile([128, 2 * T], i32)
            nc.sync.dma_start(metai, d_metai.ap())
            ctr = metai[:, 0:T]
            mb = metai[:, T : 2 * T]

            # sq_j = x^2 + y^2 + z^2 via PE column-sum (partition reduce)
            sq3 = pp.tile([3, MW], f32)
            nc.vector.tensor_tensor(sq3, posT, posT, op=Alu.mult)
            ones3 = pp.tile([3, 1], f32)
            nc.vector.memset(ones3, 1.0)
            nonesrow = pp.tile([1, MW], f32)
            nc.vector.memset(nonesrow, -1.0)
            # lhsT/rhs planes (rows 3,4 via DMA: engines cannot address
            # single partitions > 0 directly); sq is chunked so the first
            # tiles' matmuls can start before the whole sq row is done
            a5 = pp.tile([5, MW], f32)
            nc.scalar.copy(a5[0:3], posT)
            nc.sync.dma_start(a5[3:4], nonesrow)
            b5 = pp.tile([5, MW], f32)
            nc.scalar.mul(b5[0:3], posT, 2.0)
            nc.sync.dma_start(b5[4:5], nonesrow)
            sqrow = pp.tile([1, MW], f32)
            with tc.tile_pool(name="psum_prep", bufs=2, space="PSUM") as psqp:
                for part in range(0, MW, 512):
                    pe = min(part + 512, MW)
                    psq = psqp.tile([1, 512], f32, tag="sq", name="psq")
                    nc.tensor.matmul(psq[:, : pe - part], lhsT=ones3,
                                     rhs=sq3[:, part:pe], start=True, stop=True)
                    nc.scalar.copy(sqrow[:, part:pe], psq[:, : pe - part])
                    nc.sync.dma_start(a5[4:5, part:pe], sqrow[:, part:pe])
                    nc.sync.dma_start(b5[3:4, part:pe], sqrow[:, part:pe])
            ones1 = pp.tile([1, 128], f16)
            nc.vector.memset(ones1, 1.0)
            kio = pp.tile([128, K], i16)
            nc.gpsimd.iota(kio, pattern=[[1, K]], base=1, channel_multiplier=0)

            # ---- consolidated result tensors ----
            kv = pp.tile([128, T, K], f32)  # kmax of selected slots
            idxu = pp.tile([128, T, K], u16)  # local candidate index
            g16 = [pp.tile([128, T, K], f16, name=f"g16{c}", tag=f"g16{c}")
                   for c in range(3)]
            glo16 = [pp.tile([128, T, K], f16, name=f"glo16{c}", tag=f"glo16{c}")
                     for c in range(3)] if VEC_EXACT else None
            # rows above a tile's height are never written by topk/scatter;
            # give them defined (masked-out) values
            nc.gpsimd.memset(kv, NEGBIG)
            nc.gpsimd.memset(idxu, 0)
            for c in range(3):
                nc.gpsimd.memset(g16[c], 0)
                if VEC_EXACT:
                    nc.gpsimd.memset(glo16[c], 0)

            # ---- per-tile pipeline ----
            _stack = contextlib.ExitStack()
            psp = _stack.enter_context(
                tc.tile_pool(name="psum", bufs=2, space="PSUM"))
            for t_i, subs in enumerate(tiles):
                WT = max(SW[s] for s, _ in subs)
                RT = max(off + SW[s] for s, off in subs)
                psk = psp.tile([RT, WT], f32, tag="kmax", name="psk", bufs=3)
                psx = psp.tile([RT, 3, WT], f32, tag="xyz", name="psx", bufs=3)
                psxlo = (psp.tile([RT, 3, WT], f32, tag="xyzlo", name="psxlo",
                                  bufs=2) if VEC_EXACT else None)
                for s, off in subs:
                    o = int(off_slot[s])
                    R = SW[s]
                    nc.tensor.matmul(
                        psk[off : off + R, :R], lhsT=a5[:, o : o + R],
                        rhs=b5[:, o : o + R], start=True, stop=True,
                    )
                    nc.tensor.matmul(
                        psx[off : off + R].rearrange("p a b -> p (a b)"),
                        lhsT=ones1[:, :R],
                        rhs=posTf[0:1, 3 * o : 3 * (o + R)],
                        start=True, stop=True,
                    )
                    if VEC_EXACT:
                        nc.tensor.matmul(
                            psxlo[off : off + R].rearrange("p a b -> p (a b)"),
                            lhsT=ones1[:, :R],
                            rhs=posTflo[0:1, 3 * o : 3 * (o + R)],
                            start=True, stop=True,
                        )
                # kmax + diag(-inf)  (also PSUM -> SBUF)
                dg = diag if len(subs) == 1 and RT > 64 else diagp
                sbk = wp.tile([RT, WT], f32, tag="sbk", name="sbk")
                nc.vector.tensor_tensor(sbk, psk, dg[:RT, :WT], op=Alu.add)
                # fp16 candidate coordinate planes (ACT engine, cast copy)
                xh = wp.tile([RT, 3, WT], f16, tag="xh", name="xh")
                nc.scalar.copy(xh, psx)
                if VEC_EXACT:
                    # residual plane (host-computed fp16 of x - f32(f16(x)))
                    xlo = wp.tile([RT, 3, WT], f16, tag="xlo", name="xlo")
                    nc.scalar.copy(xlo, psxlo)
                # ordered top-32
                for r in range(4):
                    v8 = kv[:RT, t_i, 8 * r : 8 * r + 8]
                    i8 = idxu[:RT, t_i, 8 * r : 8 * r + 8]
                    nc.vector.max(out=v8, in_=sbk)
                    nc.vector.max_index(out=i8, in_max=v8, in_values=sbk)
                    if r < 3:
                        nc.vector.match_replace(
                            out=sbk, in_to_replace=v8, in_values=sbk,
                            imm_value=NEGBIG,
                        )
                # pads/diag slots -> negative index (scatter ignores them)
                vldneg = wp.tile([RT, K], i16, tag="vldneg", name="vldneg")
                nc.vector.tensor_scalar(
                    vldneg, kv[:RT, t_i], SANITIZE_THRESH, None, op0=Alu.is_lt
                )
                idxs = wp.tile([RT, K], i16, tag="idxs", name="idxs")
                nc.vector.scalar_tensor_tensor(
                    idxs, vldneg, -16384.0, idxu[:RT, t_i],
                    op0=Alu.mult, op1=Alu.add,
                )
                # invert: slotm[p, j] = k+1 where idxs[p, k] = j
                slotm = wp.tile([RT, WT], i16, tag="slotm", name="slotm")
                nc.gpsimd.local_scatter(
                    slotm, kio[:RT], idxs, channels=RT, num_elems=WT, num_idxs=K
                )
                nc.gpsimd.tensor_scalar(slotm, slotm, 1, None, op0=Alu.subtract)
                # payload scatters: g[p, slotm[p,j]] = coord[p, j]
                for c in range(3):
                    nc.gpsimd.local_scatter(
                        g16[c][:RT, t_i], xh[:, c], slotm,
                        channels=RT, num_elems=K, num_idxs=WT,
                    )
                if VEC_EXACT:
                    for c in range(3):
                        nc.gpsimd.local_scatter(
                            glo16[c][:RT, t_i], xlo[:, c], slotm,
                            channels=RT, num_elems=K, num_idxs=WT,
                        )
            _stack.close()

            # ---- consolidated epilogue, in two column groups so the
            # first group overlaps the last tiles' topk/scatter work ----
            kvf = kv.rearrange("p m k -> p (m k)")
            d2c = pp.tile([128, TK], f32)
            w = pp.tile([128, TK], f32)
            m1 = pp.tile([128, TK], f32)
            maskf = pp.tile([128, TK], f32)
            omask = pp.tile([128, TK], u8)
            neg1 = pp.tile([128, TK], i32)
            nc.vector.memset(neg1, -1)
            idx32 = pp.tile([128, T, K], i32)
            ctrb = pp.tile([128, T, K], i32)
            nc.vector.tensor_copy(ctrb, ctr.unsqueeze(2).to_broadcast([128, T, K]))
            oids = pp.tile([128, T, K, 2], i32)
            owvec = pp.tile([128, T, K, 4], f32)
            g1 = (2 * T) // 3
            groups = [(0, g1), (g1, T)]
            for lo, hi in groups:
                if lo >= hi:
                    continue
                ck = slice(lo * K, hi * K)
                nG = hi - lo
                nc.vector.tensor_scalar(d2c[:, ck], kvf[:, ck], -1.0, 0.0,
                                        op0=Alu.mult, op1=Alu.max)
                nc.scalar.sqrt(w[:, ck], d2c[:, ck])
                nc.vector.tensor_scalar(m1[:, ck], kvf[:, ck],
                                        -CUT_L * CUT_L, None, op0=Alu.is_le)
                nc.vector.scalar_tensor_tensor(
                    maskf[:, ck], kvf[:, ck], -CUT_U2, m1[:, ck],
                    op0=Alu.is_ge, op1=Alu.mult)
                nc.scalar.copy(omask[:, ck], maskf[:, ck])
                nc.vector.tensor_tensor(
                    owvec[:, lo:hi, :, 0],
                    w[:, ck].rearrange("p (m k) -> p m k", k=K),
                    maskf[:, ck].rearrange("p (m k) -> p m k", k=K),
                    op=Alu.mult)
                mbb = mb[:, lo:hi].unsqueeze(2).to_broadcast([128, nG, K])
                nc.vector.scalar_tensor_tensor(
                    idx32[:, lo:hi], idxu[:, lo:hi], 1.0, mbb,
                    op0=Alu.mult, op1=Alu.add)
                nc.vector.select(
                    oids[:, lo:hi, :, 0], omask[:, ck].rearrange(
                        "p (m k) -> p m k", k=K),
                    idx32[:, lo:hi], neg1[:, ck].rearrange(
                        "p (m k) -> p m k", k=K),
                )
                nc.vector.select(
                    oids[:, lo:hi, :, 1], omask[:, ck].rearrange(
                        "p (m k) -> p m k", k=K),
                    ctrb[:, lo:hi], neg1[:, ck].rearrange(
                        "p (m k) -> p m k", k=K),
                )
                for c in range(3):
                    gf = pp.tile([128, nG, K], f32, name=f"gf{c}_{lo}",
                                 tag=f"gf{c}_{lo}")
                    cb = cxyz[c][:, lo:hi].unsqueeze(2).to_broadcast([128, nG, K])
                    nc.vector.tensor_tensor(gf, g16[c][:, lo:hi], cb,
                                            op=Alu.subtract)
                    if VEC_EXACT:
                        nc.vector.tensor_tensor(gf, gf, glo16[c][:, lo:hi],
                                                op=Alu.add)
                    nc.vector.tensor_tensor(
                        owvec[:, lo:hi, :, 1 + c], gf,
                        maskf[:, ck].rearrange("p (m k) -> p m k", k=K),
                        op=Alu.mult,
                    )
                # outputs for this group (partition-major, contiguous)
                nc.sync.dma_start(d_oids.ap()[:, lo:hi], oids[:, lo:hi])
                nc.sync.dma_start(d_owvec.ap()[:, lo:hi], owvec[:, lo:hi])
                nc.scalar.dma_start(
                    d_omask.ap().rearrange("p m k -> p (m k)")[:, ck], omask[:, ck])

    nc.compile()
    return nc


def kernel(pos: np.ndarray, batch: np.ndarray, _trace: bool = False):
    from concourse.bass_utils import run_bass_kernel_spmd

    pos = np.ascontiguousarray(np.asarray(pos, dtype=np.float32))
    batch = np.asarray(batch, dtype=np.int32)
    assert pos.shape == (N, 3) and batch.shape == (N,)

    assert batch.min() >= 0 and batch.max() < NMOL
    sizes = np.bincount(batch, minlength=NMOL).astype(np.int64)
    assert sizes.sum() == N
    starts = np.zeros(NMOL + 1, np.int64)
    np.cumsum(sizes, out=starts[1:])

    order = np.argsort(-sizes, kind="stable")  # global size ranks
    slot_max = [int(sizes[order[8 * i : 8 * i + 8]].max()) for i in range(SLOTS)]
    struct_key = _structure(slot_max)
    if struct_key not in _BUILD_CACHE:
        _BUILD_CACHE[struct_key] = _build(struct_key)
    nc = _BUILD_CACHE[struct_key]

    SW, tiles = struct_key
    T = len(tiles)
    off_slot = np.concatenate([[0], np.cumsum(SW)]).astype(int)
    MW = int(off_slot[-1])
    slot_pos = {}
    for t_i, subs in enumerate(tiles):
        for s, off in subs:
            slot_pos[s] = (t_i, off)

    diag = np.zeros((128, 128), np.float32)
    np.fill_diagonal(diag, NEGBIG)
    diagp = np.zeros((128, 64), np.float32)
    for p in range(128):
        diagp[p, p % 64] = NEGBIG

    in_maps = []
    for c in range(NCORES):
        posT = np.full((3, MW), PADPOS, np.float32)
        cx = np.full((128, T), PADPOS, np.float32)
        cy = np.full((128, T), PADPOS, np.float32)
        cz = np.full((128, T), PADPOS, np.float32)
        ctr = np.full((128, T), -1, np.int32)
        mbv = np.zeros((128, T), np.int32)
        for i in range(SLOTS):
            m = int(order[8 * i + c])
            s, e = int(starts[m]), int(starts[m + 1])
            sm = e - s
            o = int(off_slot[i])
            t_i, off = slot_pos[i]
            if sm:
                posT[:, o : o + sm] = pos[s:e].T
                cx[off : off + sm, t_i] = pos[s:e, 0]
                cy[off : off + sm, t_i] = pos[s:e, 1]
                cz[off : off + sm, t_i] = pos[s:e, 2]
                ctr[off : off + sm, t_i] = np.arange(s, e, dtype=np.int32)
            mbv[off : off + SW[i], t_i] = s
        # posTf: per slot, x|y|z blocks contiguous (fp16 for the PE broadcast)
        posTfull = np.concatenate(
            [posT[:, off_slot[i] : off_slot[i + 1]].reshape(-1)
             for i in range(SLOTS)]
        )[None, :]
        posTf = posTfull.astype(np.float16)
        posTflo = (posTfull - posTf.astype(np.float32)).astype(np.float16)
        metaf = np.concatenate([cx, cy, cz], axis=1)
        metai = np.concatenate([ctr, mbv], axis=1)
        diagall = np.concatenate([diag, diagp], axis=1)
        in_maps.append(
            {"posT": posT, "posTf": posTf, "posTflo": posTflo,
             "metaf": metaf, "metai": metai, "diagall": diagall}
        )

    res = run_bass_kernel_spmd(
        nc, in_maps, core_ids=list(range(NCORES)),
        trace=_trace or bool(os.environ.get("KNN_TRACE")),
    )
    if res.exec_time_ns is not None:
        print(f"HW exec time: {res.exec_time_ns} ns")
        if res.instructions_and_trace is not None:
            print("trace:", res.instructions_and_trace[1])

    edge_index = np.empty((2, N * K), np.int32)
    edge_weight = np.empty(N * K, np.float32)
    edge_vec = np.empty((N * K, 3), np.float32)
    mask = np.empty(N * K, bool)
    rank_of = np.empty(NMOL, np.int64)
    rank_of[order] = np.arange(NMOL)
    for m in range(NMOL):
        r = int(rank_of[m])
        i, c = divmod(r, NCORES)
        s, e = int(starts[m]), int(starts[m + 1])
        sm = e - s
        if not sm:
            continue
        t_i, off = slot_pos[i]
        sl = slice(s * K, e * K)
        out = res.results[c]
        ids = out["o_ids"][off : off + sm, t_i]
        edge_index[0, sl] = ids[:, :, 0].reshape(-1)
        edge_index[1, sl] = ids[:, :, 1].reshape(-1)
        wvec = out["o_wvec"][off : off + sm, t_i]
        edge_weight[sl] = wvec[:, :, 0].reshape(-1)
        edge_vec[sl] = wvec[:, :, 1:].reshape(-1, 3)
        mask[sl] = out["o_mask"][off : off + sm, t_i].reshape(-1).astype(bool)
    return edge_index, edge_weight, edge_vec, mask


# revision 64
# speedup vs baseline: 1.0472x; 1.0472x over previous
"""Trainium2 Bass kernel for nn_Distance (radius-graph + Distance, K=32 NN).

Strategy (data-parallel over molecules, 8 NeuronCores, single SPMD NEFF):
  - batch ids are sorted -> molecules are contiguous node ranges. The 128
    molecules are sorted by size and dealt round-robin: slot i of core c
    holds the (8*i + c)-th largest molecule, so per-slot padded widths
    (multiples of 32, shared by all cores) are tight and the NEFF is
    identical across cores; only the data differs.
  - Two <=64-node slots share one 128-partition tile (rows 0-63 / 64-127);
    larger slots get their own tile.
  - kmax = -d2 via one PE matmul per sub-slot (homogeneous trick:
    lhsT = [x; y; z; -1; sq], rhs = [2x; 2y; 2z; sq; -1], fp32).
  - Ordered top-32 per row on DVE: max8 / max_index / match_replace,
    4 rounds of 8; self edges killed by adding a -1e30 diagonal constant.
  - Neighbor coordinates recovered without per-edge DMA gathers: gpsimd
    local_scatter inverts the per-row index map, then scatters fp16 hi+lo
    coordinate plane pairs (PE ones-matmul broadcasts) into the 32 output
    slots; hi + lo reconstructs the fp32 coordinate exactly.
  - Weights / masks / global ids / edge vectors via consolidated
    [128, T*32] DVE/ACT ops; host only unpads + concatenates.
"""

import os

import numpy as np

N = 8192
NMOL = 128
K = 32
NCORES = 8
SLOTS = NMOL // NCORES  # 16 size-rank bands, one slot each
CUT_U2 = 25.0  # CUTOFF_UPPER ** 2
CUT_L = 0.5  # CUTOFF_LOWER
PADPOS = 1.0e4
NEGBIG = -1.0e30
SANITIZE_THRESH = -1.0e8  # kmax below this -> padded/diag slot

_BUILD_CACHE: dict = {}
VEC_EXACT = True  # hi+lo fp16 scatter pair -> f32-exact edge_vec


def _structure(slot_max_sizes, slot_min_sizes=None):
    """Slot widths + tile packing from the per-slot max molecule size.

    Returns (SW, tiles): SW[i] = stored width of slot i (multiple of 32);
    tiles = list of ((slot, row_offset), ...) with 1 or 2 subs per tile.
    """
    if slot_min_sizes is None:
        slot_min_sizes = [0] * len(slot_max_sizes)
    san = tuple(bool(s <= 33) for s in slot_min_sizes)
    SW = []
    for sm in slot_max_sizes:
        w = max(32, -(-int(sm) // 32) * 32)
        assert w <= 128, f"molecule with {sm} nodes does not fit one tile"
        SW.append(w)
    big = [i for i, w in enumerate(SW) if w > 64]
    small = [i for i, w in enumerate(SW) if w <= 64]
    for i in small:
        SW[i] = 64  # pairable slots stored 64 wide: two stack into 128 rows
    singles = [((i, 0),) for i in big]
    pairs = []
    for j in range(0, len(small), 2):
        pair = small[j : j + 2]
        pairs.append(tuple((s, 64 * p) for p, s in enumerate(pair)))
    # interleave pairs among singles for smoother engine rotation
    tiles = []
    si, pi = 0, 0
    while si < len(singles) or pi < len(pairs):
        for _ in range(2):
            if si < len(singles):
                tiles.append(singles[si]); si += 1
        if pi < len(pairs):
            tiles.append(pairs[pi]); pi += 1
    return tuple(SW), tuple(tiles), san


def _build(struct_key):
    """Emit the SPMD Bass/Tile kernel for the given structure. Returns nc."""
    import contextlib

    import concourse.bacc as bacc
    import concourse.mybir as mybir
    import concourse.tile as tile

    SW, tiles, san = struct_key
    T = len(tiles)
    off_slot = np.concatenate([[0], np.cumsum(SW)]).astype(int)
    MW = int(off_slot[-1])
    TK = T * K

    nc = bacc.Bacc(
        "TRN2",
        target_bir_lowering=False,
        debug=False,
        num_devices=NCORES,
    )
    f32 = mybir.dt.float32
    f16 = mybir.dt.float16
    i32 = mybir.dt.int32
    i16 = mybir.dt.int16
    u16 = mybir.dt.uint16
    u8 = mybir.dt.uint8
    Alu = mybir.AluOpType

    d_posT = nc.dram_tensor("posT", [3, MW], f32, kind="ExternalInput")
    d_posTf = nc.dram_tensor("posTf", [1, 3 * MW], f16, kind="ExternalInput")
    d_posTflo = nc.dram_tensor("posTflo", [1, 3 * MW], f16, kind="ExternalInput")
    d_metaf = nc.dram_tensor("metaf", [128, 3 * T], f32, kind="ExternalInput")
    d_metai = nc.dram_tensor("metai", [128, 2 * T], i32, kind="ExternalInput")
    d_diagall = nc.dram_tensor("diagall", [128, 192], f32, kind="ExternalInput")

    d_oids = nc.dram_tensor("o_ids", [128, T, K, 2], i32, kind="ExternalOutput")
    d_owvec = nc.dram_tensor("o_wvec", [128, T, K, 4], f32, kind="ExternalOutput")
    d_omask = nc.dram_tensor("o_mask", [128, T, K], u8, kind="ExternalOutput")

    with tile.TileContext(nc) as tc:
        with (
            tc.tile_pool(name="persist", bufs=1) as pp,
            tc.tile_pool(name="work", bufs=3) as wp,
        ):
            # ---- load + prep shared planes ----
            posT = pp.tile([3, MW], f32)
            nc.sync.dma_start(posT, d_posT.ap())
            posTf = pp.tile([1, 3 * MW], f16)
            nc.sync.dma_start(posTf, d_posTf.ap())
            posTflo = pp.tile([1, 3 * MW], f16)
            if VEC_EXACT:
                nc.sync.dma_start(posTflo, d_posTflo.ap())
            diagall = pp.tile([128, 192], f32)
            nc.sync.dma_start(diagall, d_diagall.ap())
            diag = diagall[:, 0:128]
            diagp = diagall[:, 128:192]
            metaf = pp.tile([128, 3 * T], f32)
            nc.sync.dma_start(metaf, d_metaf.ap())
            cxyz = [metaf[:, c * T : (c + 1) * T] for c in range(3)]
            metai = pp.tile([128, 2 * T], i32)
            nc.sync.dma_start(metai, d_metai.ap())
            ctr = metai[:, 0:T]
            mb = metai[:, T : 2 * T]

            # sq_j = x^2 + y^2 + z^2 via PE column-sum (partition reduce)
            sq3 = pp.tile([3, MW], f32)
            nc.vector.tensor_tensor(sq3, posT, posT, op=Alu.mult)
            ones3 = pp.tile([3, 1], f32)
            nc.vector.memset(ones3, 1.0)
            nonesrow = pp.tile([1, MW], f32)
            nc.vector.memset(nonesrow, -1.0)
            # lhsT/rhs planes (rows 3,4 via DMA: engines cannot address
            # single partitions > 0 directly); sq is chunked so the first
            # tiles' matmuls can start before the whole sq row is done
            a5 = pp.tile([5, MW], f32)
            nc.scalar.copy(a5[0:3], posT)
            nc.sync.dma_start(a5[3:4], nonesrow)
            b5 = pp.tile([5, MW], f32)
            nc.scalar.mul(b5[0:3], posT, 2.0)
            nc.sync.dma_start(b5[4:5], nonesrow)
            sqrow = pp.tile([1, MW], f32)
            with tc.tile_pool(name="psum_prep", bufs=2, space="PSUM") as psqp:
                for part in range(0, MW, 512):
                    pe = min(part + 512, MW)
                    psq = psqp.tile([1, 512], f32, tag="sq", name="psq")
                    nc.tensor.matmul(psq[:, : pe - part], lhsT=ones3,
                                     rhs=sq3[:, part:pe], start=True, stop=True)
                    nc.scalar.copy(sqrow[:, part:pe], psq[:, : pe - part])
                    nc.sync.dma_start(a5[4:5, part:pe], sqrow[:, part:pe])
                    nc.sync.dma_start(b5[3:4, part:pe], sqrow[:, part:pe])
            ones1 = pp.tile([1, 128], f16)
            nc.vector.memset(ones1, 1.0)
            kio = pp.tile([128, K], i16)
            nc.gpsimd.iota(kio, pattern=[[1, K]], base=1, channel_multiplier=0)

            # ---- consolidated result tensors ----
            kv = pp.tile([128, T, K], f32)  # kmax of selected slots
            idxu = pp.tile([128, T, K], u16)  # local candidate index
            g16 = [pp.tile([128, T, K], f16, name=f"g16{c}", tag=f"g16{c}")
                   for c in range(3)]
            glo16 = [pp.tile([128, T, K], f16, name=f"glo16{c}", tag=f"glo16{c}")
                     for c in range(3)] if VEC_EXACT else None
            # rows above a tile's height are never written by topk/scatter;
            # give them defined (masked-out) values
            nc.gpsimd.memset(kv, NEGBIG)
            nc.gpsimd.memset(idxu, 0)
            for c in range(3):
                nc.gpsimd.memset(g16[c], 0)
                if VEC_EXACT:
                    nc.gpsimd.memset(glo16[c], 0)

            # ---- per-tile pipeline ----
            _stack = contextlib.ExitStack()
            psp = _stack.enter_context(
                tc.tile_pool(name="psum", bufs=2, space="PSUM"))
            for t_i, subs in enumerate(tiles):
                WT = max(SW[s] for s, _ in subs)
                RT = max(off + SW[s] for s, off in subs)
                psk = psp.tile([RT, WT], f32, tag="kmax", name="psk", bufs=3)
                psx = psp.tile([RT, 3, WT], f32, tag="xyz", name="psx", bufs=3)
                psxlo = (psp.tile([RT, 3, WT], f32, tag="xyzlo", name="psxlo",
                                  bufs=2) if VEC_EXACT else None)
                for s, off in subs:
                    o = int(off_slot[s])
                    R = SW[s]
                    nc.tensor.matmul(
                        psk[off : off + R, :R], lhsT=a5[:, o : o + R],
                        rhs=b5[:, o : o + R], start=True, stop=True,
                    )
                    nc.tensor.matmul(
                        psx[off : off + R].rearrange("p a b -> p (a b)"),
                        lhsT=ones1[:, :R],
                        rhs=posTf[0:1, 3 * o : 3 * (o + R)],
                        start=True, stop=True,
                    )
                    if VEC_EXACT:
                        nc.tensor.matmul(
                            psxlo[off : off + R].rearrange("p a b -> p (a b)"),
                            lhsT=ones1[:, :R],
                            rhs=posTflo[0:1, 3 * o : 3 * (o + R)],
                            start=True, stop=True,
                        )
                # kmax + diag(-inf)  (also PSUM -> SBUF)
                dg = diag if len(subs) == 1 and RT > 64 else diagp
                sbk = wp.tile([RT, WT], f32, tag="sbk", name="sbk")
                nc.vector.tensor_tensor(sbk, psk, dg[:RT, :WT], op=Alu.add)
                # fp16 candidate coordinate planes (ACT engine, cast copy)
                xh = wp.tile([RT, 3, WT], f16, tag="xh", name="xh")
                nc.scalar.copy(xh, psx)
                if VEC_EXACT:
                    # residual plane (host-computed fp16 of x - f32(f16(x)))
                    xlo = wp.tile([RT, 3, WT], f16, tag="xlo", name="xlo")
                    nc.scalar.copy(xlo, psxlo)
                # ordered top-32
                for r in range(4):
                    v8 = kv[:RT, t_i, 8 * r : 8 * r + 8]
                    i8 = idxu[:RT, t_i, 8 * r : 8 * r + 8]
                    nc.vector.max(out=v8, in_=sbk)
                    nc.vector.max_index(out=i8, in_max=v8, in_values=sbk)
                    if r < 3:
                        nc.vector.match_replace(
                            out=sbk, in_to_replace=v8, in_values=sbk,
                            imm_value=NEGBIG,
                        )
                if any(san[s] for s, _ in subs):
                    # pads/diag slots -> negative index (scatter ignores)
                    vldneg = wp.tile([RT, K], i16, tag="vldneg", name="vldneg")
                    nc.vector.tensor_scalar(
                        vldneg, kv[:RT, t_i], SANITIZE_THRESH, None,
                        op0=Alu.is_lt
                    )
                    idxs = wp.tile([RT, K], i16, tag="idxs", name="idxs")
                    nc.vector.scalar_tensor_tensor(
                        idxs, vldneg, -16384.0, idxu[:RT, t_i],
                        op0=Alu.mult, op1=Alu.add,
                    )
                else:
                    # every row has >= 32 real candidates: indices are
                    # already distinct and in range
                    idxs = idxu[:RT, t_i].bitcast(i16)
                # invert: slotm[p, j] = k+1 where idxs[p, k] = j
                slotm = wp.tile([RT, WT], i16, tag="slotm", name="slotm")
                nc.gpsimd.local_scatter(
                    slotm, kio[:RT], idxs, channels=RT, num_elems=WT, num_idxs=K
                )
                nc.gpsimd.tensor_scalar(slotm, slotm, 1, None, op0=Alu.subtract)
                # payload scatters: g[p, slotm[p,j]] = coord[p, j]
                for c in range(3):
                    nc.gpsimd.local_scatter(
                        g16[c][:RT, t_i], xh[:, c], slotm,
                        channels=RT, num_elems=K, num_idxs=WT,
                    )
                if VEC_EXACT:
                    for c in range(3):
                        nc.gpsimd.local_scatter(
                            glo16[c][:RT, t_i], xlo[:, c], slotm,
                            channels=RT, num_elems=K, num_idxs=WT,
                        )
            _stack.close()

            # ---- consolidated epilogue, in two column groups so the
            # first group overlaps the last tiles' topk/scatter work ----
            kvf = kv.rearrange("p m k -> p (m k)")
            d2c = pp.tile([128, TK], f32)
            w = pp.tile([128, TK], f32)
            m1 = pp.tile([128, TK], f32)
            maskf = pp.tile([128, TK], f32)
            omask = pp.tile([128, TK], u8)
            neg1 = pp.tile([128, TK], i32)
            nc.vector.memset(neg1, -1)
            idx32 = pp.tile([128, T, K], i32)
            ctrb = pp.tile([128, T, K], i32)
            nc.vector.tensor_copy(ctrb, ctr.unsqueeze(2).to_broadcast([128, T, K]))
            oids = pp.tile([128, T, K, 2], i32)
            owvec = pp.tile([128, T, K, 4], f32)
            g1 = (2 * T) // 3
            groups = [(0, g1), (g1, T)]
            for lo, hi in groups:
                if lo >= hi:
                    continue
                ck = slice(lo * K, hi * K)
                nG = hi - lo
                nc.vector.tensor_scalar(d2c[:, ck], kvf[:, ck], -1.0, 0.0,
                                        op0=Alu.mult, op1=Alu.max)
                nc.scalar.sqrt(w[:, ck], d2c[:, ck])
                nc.vector.tensor_scalar(m1[:, ck], kvf[:, ck],
                                        -CUT_L * CUT_L, None, op0=Alu.is_le)
                nc.vector.scalar_tensor_tensor(
                    maskf[:, ck], kvf[:, ck], -CUT_U2, m1[:, ck],
                    op0=Alu.is_ge, op1=Alu.mult)
                nc.scalar.copy(omask[:, ck], maskf[:, ck])
                nc.vector.tensor_tensor(
                    owvec[:, lo:hi, :, 0],
                    w[:, ck].rearrange("p (m k) -> p m k", k=K),
                    maskf[:, ck].rearrange("p (m k) -> p m k", k=K),
                    op=Alu.mult)
                mbb = mb[:, lo:hi].unsqueeze(2).to_broadcast([128, nG, K])
                nc.vector.scalar_tensor_tensor(
                    idx32[:, lo:hi], idxu[:, lo:hi], 1.0, mbb,
                    op0=Alu.mult, op1=Alu.add)
                nc.vector.select(
                    oids[:, lo:hi, :, 0], omask[:, ck].rearrange(
                        "p (m k) -> p m k", k=K),
                    idx32[:, lo:hi], neg1[:, ck].rearrange(
                        "p (m k) -> p m k", k=K),
                )
                nc.vector.select(
                    oids[:, lo:hi, :, 1], omask[:, ck].rearrange(
                        "p (m k) -> p m k", k=K),
                    ctrb[:, lo:hi], neg1[:, ck].rearrange(
                        "p (m k) -> p m k", k=K),
                )
                for c in range(3):
                    gf = pp.tile([128, nG, K], f32, name=f"gf{c}_{lo}",
                                 tag=f"gf{c}_{lo}")
                    cb = cxyz[c][:, lo:hi].unsqueeze(2).to_broadcast([128, nG, K])
                    nc.vector.tensor_tensor(gf, g16[c][:, lo:hi], cb,
                                            op=Alu.subtract)
                    if VEC_EXACT:
                        nc.vector.tensor_tensor(gf, gf, glo16[c][:, lo:hi],
                                                op=Alu.add)
                    nc.vector.tensor_tensor(
                        owvec[:, lo:hi, :, 1 + c], gf,
                        maskf[:, ck].rearrange("p (m k) -> p m k", k=K),
                        op=Alu.mult,
                    )
                # outputs for this group (partition-major, contiguous)
                nc.sync.dma_start(d_oids.ap()[:, lo:hi], oids[:, lo:hi])
                nc.sync.dma_start(d_owvec.ap()[:, lo:hi], owvec[:, lo:hi])
                nc.scalar.dma_start(
                    d_omask.ap().rearrange("p m k -> p (m k)")[:, ck], omask[:, ck])

    nc.compile()
    return nc


def kernel(pos: np.ndarray, batch: np.ndarray, _trace: bool = False):
    from concourse.bass_utils import run_bass_kernel_spmd

    pos = np.ascontiguousarray(np.asarray(pos, dtype=np.float32))
    batch = np.asarray(batch, dtype=np.int32)
    assert pos.shape == (N, 3) and batch.shape == (N,)

    assert batch.min() >= 0 and batch.max() < NMOL
    sizes = np.bincount(batch, minlength=NMOL).astype(np.int64)
    assert sizes.sum() == N
    starts = np.zeros(NMOL + 1, np.int64)
    np.cumsum(sizes, out=starts[1:])

    order = np.argsort(-sizes, kind="stable")  # global size ranks
    slot_max = [int(sizes[order[8 * i : 8 * i + 8]].max()) for i in range(SLOTS)]
    slot_min = [int(sizes[order[8 * i : 8 * i + 8]].min()) for i in range(SLOTS)]
    struct_key = _structure(slot_max, slot_min)
    if struct_key not in _BUILD_CACHE:
        _BUILD_CACHE[struct_key] = _build(struct_key)
    nc = _BUILD_CACHE[struct_key]

    SW, tiles, san = struct_key
    T = len(tiles)
    off_slot = np.concatenate([[0], np.cumsum(SW)]).astype(int)
    MW = int(off_slot[-1])
    slot_pos = {}
    for t_i, subs in enumerate(tiles):
        for s, off in subs:
            slot_pos[s] = (t_i, off)

    diag = np.zeros((128, 128), np.float32)
    np.fill_diagonal(diag, NEGBIG)
    diagp = np.zeros((128, 64), np.float32)
    for p in range(128):
        diagp[p, p % 64] = NEGBIG

    in_maps = []
    for c in range(NCORES):
        posT = np.full((3, MW), PADPOS, np.float32)
        cx = np.full((128, T), PADPOS, np.float32)
        cy = np.full((128, T), PADPOS, np.float32)
        cz = np.full((128, T), PADPOS, np.float32)
        ctr = np.full((128, T), -1, np.int32)
        mbv = np.zeros((128, T), np.int32)
        for i in range(SLOTS):
            m = int(order[8 * i + c])
            s, e = int(starts[m]), int(starts[m + 1])
            sm = e - s
            o = int(off_slot[i])
            t_i, off = slot_pos[i]
            if sm:
                posT[:, o : o + sm] = pos[s:e].T
                cx[off : off + sm, t_i] = pos[s:e, 0]
                cy[off : off + sm, t_i] = pos[s:e, 1]
                cz[off : off + sm, t_i] = pos[s:e, 2]
                ctr[off : off + sm, t_i] = np.arange(s, e, dtype=np.int32)
            mbv[off : off + SW[i], t_i] = s
        # posTf: per slot, x|y|z blocks contiguous (fp16 for the PE broadcast)
        posTfull = np.concatenate(
            [posT[:, off_slot[i] : off_slot[i + 1]].reshape(-1)
             for i in range(SLOTS)]
        )[None, :]
        posTf = posTfull.astype(np.float16)
        posTflo = (posTfull - posTf.astype(np.float32)).astype(np.float16)
        metaf = np.concatenate([cx, cy, cz], axis=1)
        metai = np.concatenate([ctr, mbv], axis=1)
        diagall = np.concatenate([diag, diagp], axis=1)
        in_maps.append(
            {"posT": posT, "posTf": posTf, "posTflo": posTflo,
             "metaf": metaf, "metai": metai, "diagall": diagall}
        )

    res = run_bass_kernel_spmd(
        nc, in_maps, core_ids=list(range(NCORES)),
        trace=_trace or bool(os.environ.get("KNN_TRACE")),
    )
    if res.exec_time_ns is not None:
        print(f"HW exec time: {res.exec_time_ns} ns")
        if res.instructions_and_trace is not None:
            print("trace:", res.instructions_and_trace[1])

    edge_index = np.empty((2, N * K), np.int32)
    edge_weight = np.empty(N * K, np.float32)
    edge_vec = np.empty((N * K, 3), np.float32)
    mask = np.empty(N * K, bool)
    rank_of = np.empty(NMOL, np.int64)
    rank_of[order] = np.arange(NMOL)
    for m in range(NMOL):
        r = int(rank_of[m])
        i, c = divmod(r, NCORES)
        s, e = int(starts[m]), int(starts[m + 1])
        sm = e - s
        if not sm:
            continue
        t_i, off = slot_pos[i]
        sl = slice(s * K, e * K)
        out = res.results[c]
        ids = out["o_ids"][off : off + sm, t_i]
        edge_index[0, sl] = ids[:, :, 0].reshape(-1)
        edge_index[1, sl] = ids[:, :, 1].reshape(-1)
        wvec = out["o_wvec"][off : off + sm, t_i]
        edge_weight[sl] = wvec[:, :, 0].reshape(-1)
        edge_vec[sl] = wvec[:, :, 1:].reshape(-1, 3)
        mask[sl] = out["o_mask"][off : off + sm, t_i].reshape(-1).astype(bool)
    return edge_index, edge_weight, edge_vec, mask
